# revision 23
# baseline (speedup 1.0000x reference)
"""Trainium2 Bass kernel for nn_ECA (attention block + residual + LayerNorm).

Reference computation (per batch b):
    qkv = x @ qkv_w.T ; q,k,v per head
    attn = softmax((q @ k.T) * sqrt(D))
    x1 = attn @ v  -> concat heads -> @ proj_w.T + proj_b
    out = LayerNorm(x + x1) * gamma + beta     # eps 1e-5

Sharding: 8 cores = 4 batches x 2 query-halves. Each core receives the full
batch's tokens ("xk", rolled so its own 1024 query tokens are rows 0:1024),
computes K/V for all 2048 keys (duplicated across the 2 cores of a batch),
attention + proj + LN for its 1024 queries. No collectives.

Precision: logits need fp32-class accuracy (softmax here is near-argmax:
logit std ~64, top-2 gap ~13 -- tf32/bf16/fp16 single-pass rounding flips
argmaxes). Native fp32 matmul runs at 1/4 rate, so Q/K/S use fp16 limb-split
matmuls: a = ah + al (fp16 high/low limbs), a.b = ah.bh + ah.bl + al.bh.

v2: the three S-limb matmuls (each K=64, half the PE contraction rows) are
restructured into two: a K=128 "stacked" pass with qdup=[qh;qh] against
kstk=[kh;kl] (computes qh.kh + qh.kl), plus a K=64 pass ql.kh. Same three
terms, 2/3 the PE fill time. kstk for odd heads is stored [kl;kh] so the
K=64 pass reads kh at partition base 64 (= ql's natural base). The halves
that land on the "wrong" partitions are moved with SBUF->SBUF DMAs (SWDGE).
Softmax uses JQ=1024 quarters (halves ACT accumulator reads + fixup work),
AV-copy runs on ACT, and proj+LN are interleaved per query-block under the
attention loop. proj bias is folded into the residual input on the host.
"""

import sys
from dataclasses import dataclass

import numpy as np

try:
    import concourse.bass  # noqa: F401
except ImportError:  # fresh dir without sitecustomize path
    sys.path.insert(0, "/opt/trn_rl_repo")


@dataclass(frozen=True)
class Cfg:
    Nk: int = 2048   # keys per core (full batch)
    Nq: int = 1024   # queries per core
    C: int = 768     # model dim (also total head dim H*D)
    H: int = 12
    D: int = 64
    lowp: str | None = None  # unused; kept for test harness compat

    @property
    def CH(self):
        return self.C // 128

    @property
    def G(self):
        return (self.H * self.D) // 128

    @property
    def TQ(self):
        return self.Nq // 128

    @property
    def TK(self):
        return self.Nk // 128

    @property
    def slabs(self):
        return self.Nk // self.Nq


def build_program(cfg: Cfg):
    import concourse.bass as bass
    import concourse.mybir as mybir
    import concourse.tile as tile

    from concourse import bacc

    f32 = mybir.dt.float32
    f16 = mybir.dt.float16
    ts = bass.ts
    Nk, Nq, C, H, D = cfg.Nk, cfg.Nq, cfg.C, cfg.H, cfg.D
    CH, G, TQ, TK = cfg.CH, cfg.G, cfg.TQ, cfg.TK
    QC = H * D
    assert QC % 128 == 0 and C % 128 == 0 and Nq % 128 == 0

    nc = bacc.Bacc("TRN2", target_bir_lowering=False, debug=False, num_devices=8)

    xh_d = nc.dram_tensor("xh16", [Nk, C], f16, kind="ExternalInput")
    xl_d = nc.dram_tensor("xl16", [Nk, C], f16, kind="ExternalInput")
    xq_d = nc.dram_tensor("xq", [Nq, C], f32, kind="ExternalInput")  # x + proj_b
    wqh_d = nc.dram_tensor("wq_hi", [C, QC], f16, kind="ExternalInput")
    wql_d = nc.dram_tensor("wq_lo", [C, QC], f16, kind="ExternalInput")
    wkh_d = nc.dram_tensor("wk_hi", [C, QC], f16, kind="ExternalInput")
    wkl_d = nc.dram_tensor("wk_lo", [C, QC], f16, kind="ExternalInput")
    wv_d = nc.dram_tensor("wv_h", [C, QC], f16, kind="ExternalInput")
    wp_d = nc.dram_tensor("wp_h", [QC, C], f16, kind="ExternalInput")
    vec_d = nc.dram_tensor("vecs", [2, C], f32, kind="ExternalInput")  # gamma, beta
    out_d = nc.dram_tensor("out", [Nq, C], f32, kind="ExternalOutput")

    J = 512              # matmul free-dim chunk (one psum bank)
    JQ = 512             # softmax quarter width (1 psum bank)
    NQS = Nk // JQ       # quarters per row (2)
    BLK = min(4, TQ)     # q-tiles per AV block

    from concourse.masks import make_identity

    with tile.TileContext(nc) as tc:
        with tc.tile_pool(name="persist", bufs=1) as persist:
            # kstk[h]: [kh;kl] for even h, [kl;kh] for odd h (so kh sits at
            # partition base r = (h*64)%128, matching ql's base for pass 2).
            kstk = [persist.tile([128, Nk], f16, name=f"kstk{h}", tag=f"kstk{h}")
                    for h in range(H)]
            idI = persist.tile([128, 128], f16, name="idI", tag="idI")
            make_identity(nc, idI[:])
            qdup = [persist.tile([128, Nq], f16, name=f"qdup{h}", tag=f"qdup{h}")
                    for h in range(H)]
            ql_t = [persist.tile([128, Nq], f16, name=f"ql{g}", tag=f"ql{g}") for g in range(G)]
            vb = [persist.tile([128, H, 65], f16, name=f"vb{t}", tag=f"vb{t}") for t in range(TK)]
            for t in range(TK):
                nc.gpsimd.memset(vb[t][:, :, 64:65], 1.0)

            # ---------------- Phase A: x^T limbs, Q^T, K^T, V ----------------
            with tc.tile_pool(name="pa_w", bufs=2) as pa_w, \
                 tc.tile_pool(name="pa_wv", bufs=1) as pa_wv, \
                 tc.tile_pool(name="pa_xh", bufs=2) as pa_xh, \
                 tc.tile_pool(name="pa_xl", bufs=1) as pa_xl, \
                 tc.tile_pool(name="pa_kl", bufs=2) as pa_kl, \
                 tc.tile_pool(name="pa_ps", bufs=4, space="PSUM") as pa_ps, \
                 tc.tile_pool(name="pa_psv", bufs=4, space="PSUM") as pa_psv:

                for slab in range(cfg.slabs):
                    # x^T fp16 limbs via cast + xbar DMA-transpose (no PE)
                    xh = pa_xh.tile([128, CH, Nq], f16, name="xh_s", tag="xh_s")
                    xl = pa_xl.tile([128, CH, Nq], f16, name="xl_s", tag="xl_s")
                    for t in range(TQ):
                        row = slice((slab * TQ + t) * 128, (slab * TQ + t + 1) * 128)
                        nc.sync.dma_start(xh[:, :, ts(t, 128)], xh_d.ap()[row, :], transpose=True)
                        nc.sync.dma_start(xl[:, :, ts(t, 128)], xl_d.ap()[row, :], transpose=True)

                    # K^T limbs (and Q^T limbs on slab 0)
                    for g in range(G):
                        h0, h1 = 2 * g, 2 * g + 1
                        klg = pa_kl.tile([128, Nq], f16, name="klg", tag="klg")
                        for which, (w_hi, w_lo) in ([("k", (wkh_d, wkl_d))] +
                                                    ([("q", (wqh_d, wql_d))] if slab == 0 else [])):
                            wgh = pa_w.tile([128, CH, 128], f16, name="wgh", tag="wgh")
                            wgl = pa_w.tile([128, CH, 128], f16, name="wgl", tag="wgl")
                            nc.sync.dma_start(wgh[:], w_hi.ap()[:, ts(g, 128)].rearrange("(c p) n -> p c n", p=128))
                            nc.sync.dma_start(wgl[:], w_lo.ap()[:, ts(g, 128)].rearrange("(c p) n -> p c n", p=128))
                            for j in range(Nq // J):
                                ps = pa_ps.tile([128, J], f32, name="ps_qk", tag="ps_qk")
                                for c in range(CH):
                                    nc.tensor.matmul(ps[:], wgh[:, c, :], xh[:, c, ts(j, J)],
                                                     start=(c == 0), stop=False)
                                    nc.tensor.matmul(ps[:], wgh[:, c, :], xl[:, c, ts(j, J)],
                                                     start=False, stop=False)
                                    nc.tensor.matmul(ps[:], wgl[:, c, :], xh[:, c, ts(j, J)],
                                                     start=False, stop=(c == CH - 1))
                                jsl = slice(j * J, (j + 1) * J)
                                if which == "k":
                                    sl = slice(slab * Nq + j * J, slab * Nq + (j + 1) * J)
                                    # hi limb -> kh at its natural partition half
                                    nc.vector.tensor_copy(kstk[h0][0:64, sl], ps[0:64, :])
                                    nc.vector.tensor_copy(kstk[h1][64:128, sl], ps[64:128, :])
                                    # lo limb -> temp (natural partitions), DMA-shifted below
                                    nc.vector.tensor_sub(klg[0:64, jsl], ps[0:64, :], kstk[h0][0:64, sl])
                                    nc.vector.tensor_sub(klg[64:128, jsl], ps[64:128, :], kstk[h1][64:128, sl])
                                else:
                                    nc.vector.tensor_copy(qdup[h0][0:64, jsl], ps[0:64, :])
                                    nc.vector.tensor_copy(qdup[h1][64:128, jsl], ps[64:128, :])
                                    nc.vector.tensor_sub(ql_t[g][0:64, jsl], ps[0:64, :], qdup[h0][0:64, jsl])
                                    nc.vector.tensor_sub(ql_t[g][64:128, jsl], ps[64:128, :], qdup[h1][64:128, jsl])
                        # kl halves into kstk's opposite partition half via PE
                        # partition-swap (identity matmul into shifted psum rows)
                        for jj in range(Nq // J):
                            jsl = slice(jj * J, (jj + 1) * J)
                            ssl = slice(slab * Nq + jj * J, slab * Nq + (jj + 1) * J)
                            ps2a = pa_ps.tile([128, J], f32, name="ps_swa", tag="ps_qk")
                            ps2b = pa_ps.tile([128, J], f32, name="ps_swb", tag="ps_qk")
                            nc.tensor.matmul(ps2a[64:128, :], idI[0:64, 0:64], klg[0:64, jsl],
                                             start=True, stop=True)
                            nc.tensor.matmul(ps2b[0:64, :], idI[64:128, 64:128], klg[64:128, jsl],
                                             start=True, stop=True)
                            nc.vector.tensor_copy(kstk[h0][64:128, ssl], ps2a[64:128, :])
                            nc.vector.tensor_copy(kstk[h1][0:64, ssl], ps2b[0:64, :])
                        if slab == 0:
                            # qdup's mirrored halves, same PE swap
                            for jj in range(Nq // J):
                                jsl = slice(jj * J, (jj + 1) * J)
                                ps2a = pa_ps.tile([128, J], f32, name="ps_swa", tag="ps_qk")
                                ps2b = pa_ps.tile([128, J], f32, name="ps_swb", tag="ps_qk")
                                nc.tensor.matmul(ps2a[64:128, :], idI[0:64, 0:64],
                                                 qdup[h0][0:64, jsl], start=True, stop=True)
                                nc.tensor.matmul(ps2b[0:64, :], idI[64:128, 64:128],
                                                 qdup[h1][64:128, jsl], start=True, stop=True)
                                nc.vector.tensor_copy(qdup[h0][64:128, jsl], ps2a[64:128, :])
                                nc.vector.tensor_copy(qdup[h1][0:64, jsl], ps2b[0:64, :])

                    # V (token-major, fp16)
                    for vc_base in range(0, QC, 384):
                        vw = min(384, QC - vc_base)
                        wvg = pa_wv.tile([128, CH, 384], f16, name="wvg", tag="wvg")
                        nc.sync.dma_start(
                            wvg[:, :, :vw],
                            wv_d.ap()[:, vc_base:vc_base + vw].rearrange("(c p) n -> p c n", p=128))
                        for t in range(TQ):
                            psv = pa_psv.tile([128, 384], f32, name="psv", tag="psv")
                            for c in range(CH):
                                nc.tensor.matmul(psv[:, :vw], xh[:, c, ts(t, 128)],
                                                 wvg[:, c, :vw],
                                                 start=(c == 0), stop=(c == CH - 1))
                            hb = vc_base // D
                            nc.vector.tensor_copy(
                                vb[slab * TQ + t][:, hb:hb + vw // D, 0:D], psv[:, :vw])

            # ---------------- Phase B: attention + proj + LN ----------------
            with tc.tile_pool(name="pc_w", bufs=1) as pc_w, \
                 tc.tile_pool(name="pb_p", bufs=2) as pb_p, \
                 tc.tile_pool(name="pb_pth", bufs=2) as pb_pth, \
                 tc.tile_pool(name="pb_st", bufs=4) as pb_st, \
                 tc.tile_pool(name="pb_rb", bufs=1) as pb_rb, \
                 tc.tile_pool(name="pb_lr", bufs=2) as pb_lr, \
                 tc.tile_pool(name="pc_sb", bufs=2) as pc_sb, \
                 tc.tile_pool(name="pc_st", bufs=3) as pc_st, \
                 tc.tile_pool(name="pb_s", bufs=6, space="PSUM") as pb_s, \
                 tc.tile_pool(name="pb_x1", bufs=2, space="PSUM") as pb_x1:
                x1t = [pc_w.tile([128, Nq], f16, name=f"x1t{g}", tag=f"x1t{g}")
                       for g in range(G)]

                # LN/proj prep
                ones = pc_w.tile([1, 128], f32, name="ones", tag="ones")
                nc.gpsimd.memset(ones[:], 1.0)
                bc = []
                for vi in range(2):
                    vrow = pc_w.tile([1, C], f32, name=f"vrow{vi}", tag=f"vrow{vi}")
                    nc.sync.dma_start(vrow[:], vec_d.ap()[vi:vi + 1, :])
                    bct = pc_w.tile([128, C], f32, name=f"bc{vi}", tag=f"bc{vi}")
                    for j in range(0, C, 512):
                        w = min(512, C - j)
                        psb = pb_x1.tile([128, 512], f32, name="psb", tag="ps_x1")
                        nc.tensor.matmul(psb[:, :w], ones[:], vrow[:, j:j + w],
                                         start=True, stop=True)
                        nc.scalar.copy(bct[:, j:j + w], psb[:, :w])
                    bc.append(bct)
                gam_bc, bet_bc = bc
                wpb = []
                for c in range(G):
                    wpc = pc_w.tile([128, C], f16, name=f"wpb{c}", tag=f"wpb{c}")
                    nc.sync.dma_start(wpc[:], wp_d.ap()[ts(c, 128), :])
                    wpb.append(wpc)
                eps_t = pc_w.tile([128, 1], f32, name="eps_t", tag="eps_t")
                nc.gpsimd.memset(eps_t[:], 1e-5)

                def emit_av(g, r, h, qb, pThb):
                    # M=65: col 64 of vb is ones -> psum row 64 = l = sum_k p
                    ps_x1 = pb_x1.tile([D + 1, BLK * 128], f32, name="ps_x1", tag="ps_x1")
                    for k in range(TK):
                        nc.tensor.matmul(ps_x1[:],
                                         vb[k][:, h, :],
                                         pThb[:, k, :, :].rearrange("p t q -> p (t q)"),
                                         start=(k == 0), stop=(k == TK - 1))
                    # 1/l via exp(-ln(l)) -- ACT table ops; DVE reciprocal on a
                    # [1,512] row costs ~2.5us (iterative divide), ACT ~0.6us each
                    ll_row = pb_lr.tile([1, BLK * 128], f32, name="ll_row", tag="lrow")
                    nc.scalar.activation(ll_row[:], ps_x1[D:D + 1, :],
                                         mybir.ActivationFunctionType.Ln)
                    rl_row = pb_lr.tile([1, BLK * 128], f32, name="rl_row", tag="lrow")
                    nc.scalar.activation(rl_row[:], ll_row[:],
                                         mybir.ActivationFunctionType.Exp, scale=-1.0)
                    rb = pb_rb.tile([D, BLK * 128], f32, name="rb", tag="rb")
                    nc.gpsimd.partition_broadcast(rb[:], rl_row[:])
                    nc.vector.tensor_mul(
                        x1t[g][r:r + D, qb * BLK * 128:(qb + 1) * BLK * 128],
                        ps_x1[0:D, :], rb[:])

                pending = None
                for qb in range(TQ // BLK):
                    for h in range(H):
                        g, r = divmod(h * D, 128)
                        for tt in range(BLK):
                            t = qb * BLK + tt
                            qd_s = qdup[h][:, ts(t, 128)]
                            ql_s = ql_t[g][r:r + D, ts(t, 128)]
                            p_t = pb_p.tile([128, Nk], f16, name="p_t", tag="p_t")
                            nm_pack = pb_st.tile([128, NQS], f32, name="nm_pack", tag="nm_pack")
                            for j2 in range(NQS):
                                ps_s = pb_s.tile([128, JQ], f32, name="ps_s", tag="ps_s")
                                for jj in range(JQ // J):
                                    sl = slice(j2 * JQ + jj * J, j2 * JQ + (jj + 1) * J)
                                    # stacked pass: qh.kh + qh.kl   (K=128)
                                    nc.tensor.matmul(ps_s[:, ts(jj, J)], qd_s, kstk[h][:, sl],
                                                     start=True, stop=False)
                                    # low-q pass: ql.kh             (K=64, rows r:r+64)
                                    nc.tensor.matmul(ps_s[:, ts(jj, J)], ql_s,
                                                     kstk[h][r:r + D, sl],
                                                     start=False, stop=True)
                                nc.vector.reduce_max(out=nm_pack[:, j2:j2 + 1], in_=ps_s[:],
                                                     axis=mybir.AxisListType.X, negate=True)
                                nc.scalar.activation(p_t[:, ts(j2, JQ)], ps_s[:],
                                                     mybir.ActivationFunctionType.Exp,
                                                     bias=nm_pack[:, j2:j2 + 1])
                            if tt == 0:
                                pThb = pb_pth.tile([128, TK, BLK, 128], f16, name="pThb", tag="pThb")
                            # local-bias exp per quarter (frees psum fast), then a
                            # cross-quarter rescale p * exp(m_j - m) on DVE. p stays
                            # un-normalized by l; the 1/l divide happens after AV
                            # (vb ones-column gives l in psum row D).
                            negm = pb_st.tile([128, 1], f32, name="negm", tag="negm")
                            nc.vector.tensor_reduce(out=negm[:], in_=nm_pack[:],
                                                    axis=mybir.AxisListType.X,
                                                    op=mybir.AluOpType.min)
                            e_p = pb_st.tile([128, NQS], f32, name="e_p", tag="e_p")
                            nc.scalar.activation(e_p[:], nm_pack[:],
                                                 mybir.ActivationFunctionType.Exp,
                                                 scale=-1.0, bias=negm[:])
                            p_n = pb_p.tile([128, Nk], f16, name="p_n", tag="p_t")
                            for j2 in range(NQS):
                                nc.vector.tensor_scalar_mul(
                                    p_n[:, ts(j2, JQ)], p_t[:, ts(j2, JQ)],
                                    e_p[:, j2:j2 + 1])
                            # blockwise transpose: pThb[p, k, tt, q] = p_n[q, k*128+p]
                            nc.sync.dma_start(pThb[:, :, tt, :], p_n[:], transpose=True)

                        if pending is not None:
                            emit_av(*pending)
                        pending = (g, r, h, qb, pThb)
                    # flush last head's AV so x1t for this qb is complete
                    if pending is not None:
                        emit_av(*pending)
                        pending = None

                    # ---- proj + residual + LayerNorm for this qb ----
                    NSTAT = 256
                    nsub = C // NSTAT
                    for tt in range(BLK):
                        t = qb * BLK + tt
                        pps = []
                        for j in range(0, C, 384):
                            w = min(384, C - j)
                            pp = pb_x1.tile([128, 384], f32, name="pp", tag="ps_x1")
                            for c in range(G):
                                nc.tensor.matmul(pp[:, :w], x1t[c][:, ts(t, 128)], wpb[c][:, j:j + w],
                                                 start=(c == 0), stop=(c == G - 1))
                            pps.append((j, w, pp))
                        xr = pc_sb.tile([128, C], f16, name="xr", tag="xrh")
                        nc.gpsimd.dma_start(xr[:], xq_d.ap()[ts(t, 128), :])
                        u = pc_sb.tile([128, C], f32, name="u", tag="u")
                        for (j, w, pp) in pps:
                            nc.vector.tensor_add(u[:, j:j + w], pp[:, :w], xr[:, j:j + w])

                        stats = pc_st.tile([128, nsub, 6], f32, name="stats", tag="stats")
                        for s in range(nsub):
                            nc.vector.bn_stats(out=stats[:, s, :], in_=u[:, ts(s, NSTAT)])
                        mv = pc_st.tile([128, 2], f32, name="mv", tag="mv")
                        nc.vector.bn_aggr(out=mv[:], in_=stats[:])
                        rstd = pc_st.tile([128, 1], f32, name="rstd", tag="rstd")
                        nc.scalar.activation(rstd[:], mv[:, 1:2],
                                             mybir.ActivationFunctionType.Sqrt, bias=eps_t[:])
                        nc.vector.reciprocal(rstd[:], rstd[:])
                        nmr = pc_st.tile([128, 1], f32, name="nmr", tag="nmr")
                        nc.vector.tensor_scalar(out=nmr[:], in0=mv[:, 0:1],
                                                scalar1=rstd[:], scalar2=-1.0,
                                                op0=mybir.AluOpType.mult,
                                                op1=mybir.AluOpType.mult)

                        of = pc_sb.tile([128, C], f32, name="of", tag="u")
                        # (u - mu)*rstd on ACT, then *gamma, +beta on GpSimd
                        nc.scalar.activation(of[:], u[:],
                                             mybir.ActivationFunctionType.Identity,
                                             scale=rstd[:], bias=nmr[:])
                        nc.gpsimd.tensor_mul(of[:], of[:], gam_bc[:])
                        nc.gpsimd.tensor_add(of[:], of[:], bet_bc[:])
                        nc.sync.dma_start(out_d.ap()[ts(t, 128), :], of[:])

    nc.compile()
    return nc


_CACHE = {}


def _get_program(cfg: Cfg):
    if cfg not in _CACHE:
        _CACHE[cfg] = build_program(cfg)
    return _CACHE[cfg]


def _split16(w):
    hi = w.astype(np.float16)
    lo = (w - hi.astype(np.float32)).astype(np.float16)
    return np.ascontiguousarray(hi), np.ascontiguousarray(lo)


def make_in_maps(x, qkv_w, proj_w, proj_b, ln_gamma, ln_beta, cfg: Cfg):
    """Host-side shard prep. Returns list of 8 in_maps."""
    C = cfg.C
    B = x.shape[0]
    wq_t = np.ascontiguousarray((qkv_w[0:C] * np.float32(cfg.D ** 0.5)).T)
    wk_t = np.ascontiguousarray(qkv_w[C:2 * C].T)
    wv_t = np.ascontiguousarray(qkv_w[2 * C:3 * C].T)
    wp_t = np.ascontiguousarray(proj_w.T)
    wq_hi, wq_lo = _split16(wq_t)
    wk_hi, wk_lo = _split16(wk_t)
    wv_h = wv_t.astype(np.float16)
    wp_h = wp_t.astype(np.float16)
    vecs = np.ascontiguousarray(np.stack([ln_gamma, ln_beta]).astype(np.float32))
    pb32 = proj_b.astype(np.float32)[None, :]
    in_maps = []
    for core in range(8):
        b, half = core // 2, core % 2
        b = min(b, B - 1)
        xb = np.asarray(x[b], dtype=np.float32)
        if half == 0:
            xkc = np.ascontiguousarray(xb)
        else:
            xkc = np.ascontiguousarray(np.concatenate([xb[cfg.Nq:], xb[:cfg.Nq]], axis=0))
        xh16, xl16 = _split16(xkc)
        in_maps.append({"xh16": xh16, "xl16": xl16,
                        "xq": np.ascontiguousarray(xkc[:cfg.Nq] + pb32),
                        "wq_hi": wq_hi, "wq_lo": wq_lo,
                        "wk_hi": wk_hi, "wk_lo": wk_lo, "wv_h": wv_h,
                        "wp_h": wp_h, "vecs": vecs})
    return in_maps


def kernel(x, qkv_w, proj_w, proj_b, ln_gamma, ln_beta):
    from concourse.bass_utils import run_bass_kernel_spmd

    cfg = Cfg()
    nc = _get_program(cfg)
    x = np.asarray(x, dtype=np.float32)
    in_maps = make_in_maps(x, np.asarray(qkv_w, np.float32), np.asarray(proj_w, np.float32),
                           np.asarray(proj_b, np.float32), np.asarray(ln_gamma, np.float32),
                           np.asarray(ln_beta, np.float32), cfg)
    res = run_bass_kernel_spmd(nc, in_maps, core_ids=list(range(8)))
    B, N, C = x.shape
    out = np.empty((B, N, C), dtype=np.float32)
    for core in range(8):
        b, half = core // 2, core % 2
        out[b, half * cfg.Nq:(half + 1) * cfg.Nq] = res.results[core]["out"]
    return out
